# revision 1
# baseline (speedup 1.0000x reference)
"""Trainium2 Bass kernel for Conv2dWeightModulate (no style).

The reference computes an equalized-lr + demodulated 3x3 conv:
    w = weight * C_EQ;  w *= rsqrt(sum(w^2, (I,K,K)) + eps);  out = conv2d(x, w, pad=1)

The tiny weight normalization runs on host (numpy); the conv runs on 8
NeuronCores, data-parallel over the batch (2 images per core).

Host-side data layout: x is cast to bf16 and split by row parity into
xP[b, c, p, h2, w] (= x[b, c, 2*h2+p, w]) so every DMA reads long
contiguous spans; the device likewise writes a parity-split fp32 output
that the host re-interleaves.

Device kernel layout (per core):
  x is stored in SBUF parity-interleaved: partitions 0-63 hold the 64
  channels of even image rows, partitions 64-127 the odd rows, with each
  row padded to 258 columns (zero borders give the conv its padding).
  Chunk column s of a block with row base R holds:
      half A (parts 0:64):   x row R + 2(s-1)
      half B (parts 64:128): x row R + 2s - 1
  so chunk s aligns x rows (2j, 2j+1) vertically.  A 3x3 conv then becomes,
  per pair of same-parity output rows (one 512-wide matmul free dim):
      - even rows: K=128 matmul (taps kh=1+kh=2) x3 kw  +  K=64 (kh=0) x3
      - odd rows:  K=128 matmul (taps kh=0+kh=1) x3 kw  +  K=64 (kh=2) x3
  Adjacent row-pairs are col-tiled (tile_position via PSUM base partition
  64) so the pair runs concurrently on disjoint PE column groups; the
  K=64 leftovers of even/odd chunks land on disjoint PE quadrants and run
  4-way concurrent.  Accumulation is fp32 in PSUM; outputs staged through
  SBUF in 32-row groups and DMAed out as fp32.
"""

import numpy as np

IN_F = 64
OUT_F = 64
KS = 3
EPS = 1e-05
C_EQ = 1.0 / np.sqrt(IN_F * KS * KS)

B_FULL = 16
H_FULL = 256
W = 256
N_CORES = 8
CW = W + 2  # padded row width


def build_nc(bpc, h, block=64, out_bf16=False):
    """Build the per-core Bass program: bpc images of [64, h, 256] each."""
    from concourse import bacc
    import concourse.mybir as mybir
    from concourse.tile import TileContext

    assert h % block == 0 and block % 32 == 0
    nblk = h // block
    ngrp = block // 32  # 32-row output staging groups per block
    sch = block // 2 + 2  # chunk columns per x tile
    f32 = mybir.dt.float32
    bf16 = mybir.dt.bfloat16

    nc = bacc.Bacc("TRN2", target_bir_lowering=False, debug=False)
    x = nc.dram_tensor("x", [bpc, IN_F, 2, h // 2, CW], bf16, kind="ExternalInput")
    wp = nc.dram_tensor("wpack", [128, 9, 64], bf16, kind="ExternalInput")
    odt = bf16 if out_bf16 else f32
    out = nc.dram_tensor("out", [bpc, OUT_F, 2, h // 2, W], odt, kind="ExternalOutput")

    # out h2 index decomposed as 16*hg + 4*uu + 2*uh + up
    outr = out.ap().rearrange("b c p (hg uu uh up) w -> b c p hg uu uh up w", uu=4, uh=2, up=2)

    with TileContext(nc) as tc:
        with (
            tc.tile_pool(name="xp", bufs=4) as xpool,
            tc.tile_pool(name="wpool", bufs=1) as wpool,
            tc.tile_pool(name="st", bufs=7) as spool,
            tc.tile_pool(name="ps", bufs=2, space="PSUM") as ppool,
        ):
            wt = wpool.tile([128, 9, 64], bf16)
            nc.sync.dma_start(out=wt[:], in_=wp.ap())
            for b in range(bpc):
                for blk in range(nblk):
                    R = blk * block
                    h0 = R // 2
                    xt = xpool.tile([128, sch, CW], bf16, tag="xt")
                    # host pre-pads rows to 258 with zero borders, so every
                    # transfer is one contiguous span per channel
                    # half A <- even x rows R .. R+block (chunks 1..sch-1)
                    # half B <- odd x rows R-1 .. R+block-1 (chunks 0..sch-2)
                    # each issued as two DMAs so separate queues split the load
                    hm = 8 if (b == 0 and blk == 0) else sch // 2
                    if blk == nblk - 1:
                        a_lo, a_hi = h0, h0 + sch - 2
                        nc.gpsimd.memset(xt[0:64, sch - 1, :], 0.0)
                    else:
                        a_lo, a_hi = h0, h0 + sch - 1
                    nc.sync.dma_start(
                        out=xt[0:64, 1 : 1 + hm, :],
                        in_=x.ap()[b, :, 0, a_lo : a_lo + hm, :],
                    )
                    nc.sync.dma_start(
                        out=xt[0:64, 1 + hm : 1 + (a_hi - a_lo), :],
                        in_=x.ap()[b, :, 0, a_lo + hm : a_hi, :],
                    )
                    if blk == 0:
                        nc.gpsimd.memset(xt[64:128, 0, :], 0.0)
                        b_s, b_lo, b_hi = 1, 0, sch - 2
                    else:
                        b_s, b_lo, b_hi = 0, h0 - 1, h0 + sch - 2
                    nc.sync.dma_start(
                        out=xt[64:128, b_s : b_s + hm, :],
                        in_=x.ap()[b, :, 1, b_lo : b_lo + hm, :],
                    )
                    nc.sync.dma_start(
                        out=xt[64:128, b_s + hm : b_s + (b_hi - b_lo), :],
                        in_=x.ap()[b, :, 1, b_lo + hm : b_hi, :],
                    )
                    for g in range(ngrp):
                        hg = (h0 + 16 * g) // 16
                        stE = spool.tile([128, 4, 2, W], odt, tag="stE")
                        stO = spool.tile([128, 4, 2, W], odt, tag="stO")
                        for uu in range(4):
                            r0 = R + 32 * g + 8 * uu
                            s0 = (r0 - R) // 2 + 1  # A-chunk of x row r0
                            psE1 = ppool.tile([128, 2, W], f32, tag="psE1")
                            psE2 = ppool.tile([128, 2, W], f32, tag="psE2")
                            psO1 = ppool.tile([128, 2, W], f32, tag="psO1")
                            psO2 = ppool.tile([128, 2, W], f32, tag="psO2")
                            # E mains: out rows (r0, r0+2 | r0+4, r0+6), taps kh=1,2
                            for kw in range(3):
                                st_ = kw == 0
                                nc.tensor.matmul(
                                    psE1[0:64], wt[:, kw, :],
                                    xt[:, s0 : s0 + 2, kw : kw + W],
                                    start=st_, stop=False,
                                )
                                nc.tensor.matmul(
                                    psE2[64:128], wt[:, kw, :],
                                    xt[:, s0 + 2 : s0 + 4, kw : kw + W],
                                    start=st_, stop=False,
                                )
                            # O mains: out rows (r0+1, r0+3 | r0+5, r0+7), taps kh=0,1
                            for kw in range(3):
                                st_ = kw == 0
                                nc.tensor.matmul(
                                    psO1[0:64], wt[:, 3 + kw, :],
                                    xt[:, s0 : s0 + 2, kw : kw + W],
                                    start=st_, stop=False,
                                )
                                nc.tensor.matmul(
                                    psO2[64:128], wt[:, 3 + kw, :],
                                    xt[:, s0 + 2 : s0 + 4, kw : kw + W],
                                    start=st_, stop=False,
                                )
                            # leftovers (4-way concurrent PE quadrants):
                            # E: tap kh=0 from half B; O: tap kh=2 from half A
                            for kw in range(3):
                                sp_ = kw == 2
                                nc.tensor.matmul(
                                    psE1[0:64], wt[64:128, 6 + kw, :],
                                    xt[64:128, s0 - 1 : s0 + 1, kw : kw + W],
                                    start=False, stop=sp_,
                                )
                                nc.tensor.matmul(
                                    psE2[64:128], wt[64:128, 6 + kw, :],
                                    xt[64:128, s0 + 1 : s0 + 3, kw : kw + W],
                                    start=False, stop=sp_,
                                )
                                nc.tensor.matmul(
                                    psO1[0:64], wt[0:64, 6 + kw, :],
                                    xt[0:64, s0 + 1 : s0 + 3, kw : kw + W],
                                    start=False, stop=sp_,
                                )
                                nc.tensor.matmul(
                                    psO2[64:128], wt[0:64, 6 + kw, :],
                                    xt[0:64, s0 + 3 : s0 + 5, kw : kw + W],
                                    start=False, stop=sp_,
                                )
                            nc.scalar.copy(stE[0:64, uu], psE1[0:64])
                            nc.scalar.copy(stE[64:128, uu], psE2[64:128])
                            nc.vector.tensor_copy(out=stO[0:64, uu], in_=psO1[0:64])
                            nc.vector.tensor_copy(out=stO[64:128, uu], in_=psO2[64:128])
                        # group output DMAs all on gpsimd: sync stays free
                        # to prefetch the next block's x tiles
                        nc.gpsimd.dma_start(
                            out=outr[b, :, 0, hg, :, 0, :, :], in_=stE[0:64]
                        )
                        nc.gpsimd.dma_start(
                            out=outr[b, :, 0, hg, :, 1, :, :], in_=stE[64:128]
                        )
                        nc.gpsimd.dma_start(
                            out=outr[b, :, 1, hg, :, 0, :, :], in_=stO[0:64]
                        )
                        nc.gpsimd.dma_start(
                            out=outr[b, :, 1, hg, :, 1, :, :], in_=stO[64:128]
                        )
    nc.compile()
    return nc


def normalize_weight(weight):
    """Host-side equalized-lr + demodulation of the [O,I,3,3] weight."""
    w = np.asarray(weight, dtype=np.float32) * np.float32(C_EQ)
    sigma_inv = 1.0 / np.sqrt(
        np.sum((w * w).astype(np.float32), axis=(1, 2, 3), keepdims=True) + EPS
    )
    return (w * sigma_inv.astype(np.float32)).astype(np.float32)


def pack_weights(w_norm):
    """Pack normalized [O,I,kh,kw] weights into the [128, 9, 64] SBUF image.

    Column group g = kw for the even-row mains (rows 0:64 <- kh=1,
    rows 64:128 <- kh=2), g = 3+kw for odd-row mains (kh=0 / kh=1),
    g = 6+kw for the leftovers (rows 0:64 <- kh=2, rows 64:128 <- kh=0).
    Each [64, 64] slice is wT = w[:, :, kh, kw].T (contraction dim first).
    """
    wt = np.transpose(w_norm, (2, 3, 1, 0))  # [kh, kw, in, out]
    wpack = np.zeros((128, 9, 64), dtype=np.float32)
    for kw in range(3):
        wpack[0:64, kw] = wt[1, kw]
        wpack[64:128, kw] = wt[2, kw]
        wpack[0:64, 3 + kw] = wt[0, kw]
        wpack[64:128, 3 + kw] = wt[1, kw]
        wpack[0:64, 6 + kw] = wt[2, kw]
        wpack[64:128, 6 + kw] = wt[0, kw]
    return wpack


_NC_CACHE = {}


def _get_nc(bpc, h, block=64, out_bf16=False):
    key = (bpc, h, block, out_bf16)
    if key not in _NC_CACHE:
        _NC_CACHE[key] = build_nc(bpc, h, block, out_bf16)
    return _NC_CACHE[key]


def split_parity(x_f32):
    """[b, c, h, w] f32 -> bf16 [b, c, 2, h//2, w+2]: row parity split plus
    zero border columns (p=0 even rows, p=1 odd rows)."""
    import ml_dtypes

    b, c, h, w = x_f32.shape
    xb = x_f32.astype(ml_dtypes.bfloat16)
    xP = np.zeros((b, c, 2, h // 2, w + 2), dtype=ml_dtypes.bfloat16)
    xP[:, :, 0, :, 1:-1] = xb[:, :, 0::2]
    xP[:, :, 1, :, 1:-1] = xb[:, :, 1::2]
    return xP


def merge_parity(outP):
    """[b, c, 2, h2, w] (any float dtype) -> fp32 [b, c, 2*h2, w]."""
    b, c, _, h2, w = outP.shape
    out = np.empty((b, c, 2 * h2, w), dtype=np.float32)
    out[:, :, 0::2] = outP[:, :, 0]
    out[:, :, 1::2] = outP[:, :, 1]
    return out


def kernel(x, weight):
    import ml_dtypes
    from concourse import bass_utils

    x = np.asarray(x, dtype=np.float32)
    weight = np.asarray(weight, dtype=np.float32)
    assert x.shape == (B_FULL, IN_F, H_FULL, W), x.shape

    xP = split_parity(x)
    wpack = pack_weights(normalize_weight(weight)).astype(ml_dtypes.bfloat16)
    bpc = B_FULL // N_CORES
    nc = _get_nc(bpc, H_FULL)
    in_maps = [
        {"x": xP[i * bpc : (i + 1) * bpc], "wpack": wpack} for i in range(N_CORES)
    ]
    res = bass_utils.run_bass_kernel_spmd(nc, in_maps, core_ids=list(range(N_CORES)))
    return np.concatenate([merge_parity(r["out"]) for r in res.results], axis=0)



# revision 4
# speedup vs baseline: 1.1068x; 1.1068x over previous
"""Trainium2 Bass kernel for Conv2dWeightModulate (no style).

The reference computes an equalized-lr + demodulated 3x3 conv:
    w = weight * C_EQ;  w *= rsqrt(sum(w^2, (I,K,K)) + eps);  out = conv2d(x, w, pad=1)

The tiny weight normalization runs on host (numpy); the conv runs on 8
NeuronCores, data-parallel over the batch (2 images per core).

Host-side data layout: x is cast to bf16 and split by row parity into
xP[b, c, p, h2, w] (= x[b, c, 2*h2+p, w]) so every DMA reads long
contiguous spans; the device writes bf16 output in a staging-matched
layout that the host re-interleaves and upcasts to fp32.

Device kernel layout (per core):
  x is stored in SBUF parity-interleaved: partitions 0-63 hold the 64
  channels of even image rows, partitions 64-127 the odd rows, with each
  row padded to 258 columns (zero borders give the conv its padding).
  Chunk column s of a block with row base R holds:
      half A (parts 0:64):   x row R + 2(s-1)
      half B (parts 64:128): x row R + 2s - 1

  Each "uu" unit covers 8 output rows via two PSUM banks that mix both
  row parities so every main matmul is a full-array K=128 x M=128 op:
      ps1[0:64]   = E rows (r0,   r0+2)   ps1[64:128] = O rows (r0+1, r0+3)
      ps2[0:64]   = O rows (r0+5, r0+7)   ps2[64:128] = E rows (r0+4, r0+6)
  (ps2 parity-flipped so the four K=64 leftover matmuls land on four
  distinct PE quadrants and run concurrently.)  Mains: 3 kw-taps x 2
  banks, one M=128 matmul each (both parities share the moving data);
  leftovers: 3 kw x 4 quadrant matmuls.  Per group of 4 uu all mains
  are emitted before all leftovers to minimize row-group transitions.
  Accumulation is fp32 in PSUM; full-width copies stage bf16 results
  through SBUF and two half-group DMAs write contiguous 512 KB spans.
"""

import numpy as np

IN_F = 64
OUT_F = 64
KS = 3
EPS = 1e-05
C_EQ = 1.0 / np.sqrt(IN_F * KS * KS)

B_FULL = 16
H_FULL = 256
W = 256
N_CORES = 8
CW = W + 2  # padded row width


def build_nc(bpc, h, block=64):
    """Build the per-core Bass program: bpc images of [64, h, 256] each."""
    from concourse import bacc
    import concourse.mybir as mybir
    from concourse.tile import TileContext

    assert h % block == 0 and block % 32 == 0
    nblk = h // block
    ngrp = block // 32  # 32-row groups per block
    sch = block // 2 + 2  # chunk columns per x tile
    f32 = mybir.dt.float32
    bf16 = mybir.dt.bfloat16

    nc = bacc.Bacc("TRN2", target_bir_lowering=False, debug=False)
    x = nc.dram_tensor("x", [bpc, IN_F, 2, h // 2, CW], bf16, kind="ExternalInput")
    wm = nc.dram_tensor("wm", [128, 6, 128], bf16, kind="ExternalInput")
    wl = nc.dram_tensor("wl", [128, 3, 64], bf16, kind="ExternalInput")
    # [b, group, half(2 uu each), partition, 2048 bf16] — each half-group DMA
    # writes one fully contiguous 512 KB span
    out = nc.dram_tensor(
        "out", [bpc, h // 32, 2, 128, 2048], bf16, kind="ExternalOutput"
    )

    with TileContext(nc) as tc:
        with (
            tc.tile_pool(name="xp", bufs=4) as xpool,
            tc.tile_pool(name="wpool", bufs=1) as wpool,
            tc.tile_pool(name="st", bufs=3) as spool,
            tc.tile_pool(name="ps", bufs=4, space="PSUM") as ppool,
        ):
            wmt = wpool.tile([128, 6, 128], bf16, tag="wm")
            wlt = wpool.tile([128, 3, 64], bf16, tag="wl")
            nc.sync.dma_start(out=wmt[:], in_=wm.ap())
            nc.sync.dma_start(out=wlt[:], in_=wl.ap())
            for b in range(bpc):
                for blk in range(nblk):
                    R = blk * block
                    h0 = R // 2
                    xt = xpool.tile([128, sch, CW], bf16, tag="xt")
                    # host pre-pads rows to 258 with zero borders, so every
                    # transfer is one contiguous span per channel.
                    # half A <- even x rows (chunks 1..sch-1)
                    # half B <- odd x rows (chunks 0..sch-2)
                    if blk == nblk - 1:
                        na = sch - 2  # A chunks 1..sch-2; chunk sch-1 is zero
                        nc.gpsimd.memset(xt[0:64, sch - 1, :], 0.0)
                    else:
                        na = sch - 1
                    if blk == 0:
                        nc.gpsimd.memset(xt[64:128, 0, :], 0.0)
                        b_s, b_lo = 1, 0
                        nb = sch - 2  # B chunks 1..sch-2
                    else:
                        b_s, b_lo = 0, h0 - 1
                        nb = sch - 1  # B chunks 0..sch-2
                    # interleave A/B sub-loads so the first matmuls (which
                    # need both parity halves) unblock as early as possible
                    if b == 0 and blk == 0:
                        cuts = [6, 14]
                    else:
                        cuts = [sch // 2]
                    splits = [0] + cuts + [max(na, nb)]
                    for lo, hi in zip(splits[:-1], splits[1:]):
                        alo, ahi = min(lo, na), min(hi, na)
                        if ahi > alo:
                            nc.sync.dma_start(
                                out=xt[0:64, 1 + alo : 1 + ahi, :],
                                in_=x.ap()[b, :, 0, h0 + alo : h0 + ahi, :],
                            )
                        blo, bhi = min(lo, nb), min(hi, nb)
                        if bhi > blo:
                            nc.sync.dma_start(
                                out=xt[64:128, b_s + blo : b_s + bhi, :],
                                in_=x.ap()[b, :, 1, b_lo + blo : b_lo + bhi, :],
                            )
                    for g in range(ngrp):
                        G = (R + 32 * g) // 32
                        st = spool.tile([128, 4, 2, 2, W], bf16, tag="st")
                        ps = [
                            (
                                ppool.tile([128, 2, W], f32, tag="ps1", name="ps1"),
                                ppool.tile([128, 2, W], f32, tag="ps2", name="ps2"),
                            )
                            for _ in range(4)
                        ]
                        # mains: full-array K=128 M=128 matmuls, both banks
                        for uu in range(4):
                            s0 = 16 * g + 4 * uu + 1
                            ps1, ps2 = ps[uu]
                            for kw in range(3):
                                nc.tensor.matmul(
                                    ps1[:], wmt[:, kw, :],
                                    xt[:, s0 : s0 + 2, kw : kw + W],
                                    start=kw == 0, stop=False,
                                )
                            for kw in range(3):
                                nc.tensor.matmul(
                                    ps2[:], wmt[:, 3 + kw, :],
                                    xt[:, s0 + 2 : s0 + 4, kw : kw + W],
                                    start=kw == 0, stop=False,
                                )
                        # leftovers: 4 distinct PE quadrants per kw-slot
                        for uu in range(4):
                            s0 = 16 * g + 4 * uu + 1
                            ps1, ps2 = ps[uu]
                            for kw in range(3):
                                sp_ = kw == 2
                                nc.tensor.matmul(
                                    ps1[0:64], wlt[64:128, kw, :],
                                    xt[64:128, s0 - 1 : s0 + 1, kw : kw + W],
                                    start=False, stop=sp_,
                                )
                                nc.tensor.matmul(
                                    ps1[64:128], wlt[0:64, kw, :],
                                    xt[0:64, s0 + 1 : s0 + 3, kw : kw + W],
                                    start=False, stop=sp_,
                                )
                                nc.tensor.matmul(
                                    ps2[64:128], wlt[64:128, kw, :],
                                    xt[64:128, s0 + 1 : s0 + 3, kw : kw + W],
                                    start=False, stop=sp_,
                                )
                                nc.tensor.matmul(
                                    ps2[0:64], wlt[0:64, kw, :],
                                    xt[0:64, s0 + 3 : s0 + 5, kw : kw + W],
                                    start=False, stop=sp_,
                                )
                        # full-width PSUM->SBUF copies, split across engines
                        for uu in range(4):
                            ps1, ps2 = ps[uu]
                            nc.scalar.copy(st[:, uu, 0], ps1[:])
                            nc.vector.tensor_copy(out=st[:, uu, 1], in_=ps2[:])
                        nc.gpsimd.dma_start(out=out.ap()[b, G, 0], in_=st[:, 0:2])
                        nc.gpsimd.dma_start(out=out.ap()[b, G, 1], in_=st[:, 2:4])
    nc.compile()
    return nc


def normalize_weight(weight):
    """Host-side equalized-lr + demodulation of the [O,I,3,3] weight."""
    w = np.asarray(weight, dtype=np.float32) * np.float32(C_EQ)
    sigma_inv = 1.0 / np.sqrt(
        np.sum((w * w).astype(np.float32), axis=(1, 2, 3), keepdims=True) + EPS
    )
    return (w * sigma_inv.astype(np.float32)).astype(np.float32)


def pack_weights(w_norm):
    """Pack normalized [O,I,kh,kw] weights into wm [128,6,128] / wl [128,3,64].

    wm column set kw is the merged main tap for bank ps1 (stationary rows
    0:64 act on even x rows / rows 64:128 on odd x rows; output cols 0:64
    are E rows, 64:128 O rows); set 3+kw is the parity-flipped bank ps2.
    wl holds the leftover taps: rows 0:64 = kh=2 (O leftover, moving half
    A), rows 64:128 = kh=0 (E leftover, moving half B).
    """
    wt = np.transpose(w_norm, (2, 3, 1, 0))  # [kh, kw, in, out]
    wm = np.zeros((128, 6, 128), dtype=np.float32)
    wl = np.zeros((128, 3, 64), dtype=np.float32)
    for kw in range(3):
        wm[0:64, kw, 0:64] = wt[1, kw]
        wm[64:128, kw, 0:64] = wt[2, kw]
        wm[0:64, kw, 64:128] = wt[0, kw]
        wm[64:128, kw, 64:128] = wt[1, kw]
        wm[0:64, 3 + kw, 0:64] = wt[0, kw]
        wm[64:128, 3 + kw, 0:64] = wt[1, kw]
        wm[0:64, 3 + kw, 64:128] = wt[1, kw]
        wm[64:128, 3 + kw, 64:128] = wt[2, kw]
        wl[0:64, kw] = wt[2, kw]
        wl[64:128, kw] = wt[0, kw]
    return wm, wl


_NC_CACHE = {}


def _get_nc(bpc, h, block=64):
    key = (bpc, h, block)
    if key not in _NC_CACHE:
        _NC_CACHE[key] = build_nc(bpc, h, block)
    return _NC_CACHE[key]


def split_parity(x_f32):
    """[b, c, h, w] f32 -> bf16 [b, c, 2, h//2, w+2]: row parity split plus
    zero border columns (p=0 even rows, p=1 odd rows)."""
    import ml_dtypes

    b, c, h, w = x_f32.shape
    xb = x_f32.astype(ml_dtypes.bfloat16)
    xP = np.zeros((b, c, 2, h // 2, w + 2), dtype=ml_dtypes.bfloat16)
    xP[:, :, 0, :, 1:-1] = xb[:, :, 0::2]
    xP[:, :, 1, :, 1:-1] = xb[:, :, 1::2]
    return xP


def decode_out(o, h):
    """Device out [bpc, h//32, 2, 128, 2048] bf16 -> fp32 [bpc, 64, h, 256].

    Element (b, G, half, ptop*64+ch, ((uuh*2+bank)*2+j)*256+w) is output
    row 32G + 16half + 8uuh + off, where off = 2j+ptop for bank 0
    (ps1: E rows on partitions 0:64, O on 64:128) and 2j+5-ptop for
    bank 1 (ps2 parity-flipped).
    """
    bpc = o.shape[0]
    nG = h // 32
    ov = np.asarray(o).reshape(bpc, nG, 2, 2, 64, 2, 2, 2, W)
    # dims: (b, G, half, ptop, ch, uuh, bank, j, w)
    outf = np.empty((bpc, OUT_F, h, W), dtype=np.float32)
    outv = outf.reshape(bpc, OUT_F, nG, 2, 2, 8, W)  # (b,ch,G,half,uuh,off,w)
    for ptop in range(2):
        for bank in range(2):
            for j in range(2):
                off = 2 * j + (ptop if bank == 0 else 5 - ptop)
                outv[:, :, :, :, :, off, :] = ov[
                    :, :, :, ptop, :, :, bank, j, :
                ].transpose(0, 3, 1, 2, 4, 5)
    return outf


def kernel(x, weight):
    import ml_dtypes
    from concourse import bass_utils

    x = np.asarray(x, dtype=np.float32)
    weight = np.asarray(weight, dtype=np.float32)
    assert x.shape == (B_FULL, IN_F, H_FULL, W), x.shape

    xP = split_parity(x)
    wm, wl = pack_weights(normalize_weight(weight))
    wm = wm.astype(ml_dtypes.bfloat16)
    wl = wl.astype(ml_dtypes.bfloat16)
    bpc = B_FULL // N_CORES
    nc = _get_nc(bpc, H_FULL)
    in_maps = [
        {"x": xP[i * bpc : (i + 1) * bpc], "wm": wm, "wl": wl}
        for i in range(N_CORES)
    ]
    res = bass_utils.run_bass_kernel_spmd(nc, in_maps, core_ids=list(range(N_CORES)))
    return np.concatenate(
        [decode_out(r["out"], H_FULL) for r in res.results], axis=0
    )


# revision 6
# speedup vs baseline: 1.1455x; 1.0349x over previous
"""Trainium2 Bass kernel for Conv2dWeightModulate (no style).

The reference computes an equalized-lr + demodulated 3x3 conv:
    w = weight * C_EQ;  w *= rsqrt(sum(w^2, (I,K,K)) + eps);  out = conv2d(x, w, pad=1)

The tiny weight normalization runs on host (numpy); the conv runs on 8
NeuronCores, data-parallel over the batch (2 images per core).

Host-side data layout: x is cast to bf16 and split by row parity into
xP[b, c, p, h2, w] (= x[b, c, 2*h2+p, w]) so every DMA reads long
contiguous spans; the device writes bf16 output in a staging-matched
layout that the host re-interleaves and upcasts to fp32.

Device kernel layout (per core):
  x is stored in SBUF parity-interleaved: partitions 0-63 hold the 64
  channels of even image rows, partitions 64-127 the odd rows, with each
  row padded to 258 columns (zero borders give the conv its padding).
  Chunk column s of a block with row base R holds:
      half A (parts 0:64):   x row R + 2(s-1)
      half B (parts 64:128): x row R + 2s - 1

  Each "uu" unit covers 8 output rows via two PSUM banks that mix both
  row parities so every main matmul is a full-array K=128 x M=128 op:
      ps1[0:64]   = E rows (r0,   r0+2)   ps1[64:128] = O rows (r0+1, r0+3)
      ps2[0:64]   = O rows (r0+5, r0+7)   ps2[64:128] = E rows (r0+4, r0+6)
  (ps2 parity-flipped so the four K=64 leftover matmuls land on four
  distinct PE quadrants and run concurrently.)  Mains: 3 kw-taps x 2
  banks, one M=128 matmul each (both parities share the moving data);
  leftovers: 3 kw x 4 quadrant matmuls.  Per group of 4 uu all mains
  are emitted before all leftovers to minimize row-group transitions.
  Accumulation is fp32 in PSUM; full-width copies stage bf16 results
  through SBUF and two half-group DMAs write contiguous 512 KB spans.
"""

import numpy as np

IN_F = 64
OUT_F = 64
KS = 3
EPS = 1e-05
C_EQ = 1.0 / np.sqrt(IN_F * KS * KS)

B_FULL = 16
H_FULL = 256
W = 256
N_CORES = 8
CW = W + 2  # padded row width


def build_nc(bpc, h, block=64):
    """Build the per-core Bass program: bpc images of [64, h, 256] each."""
    from concourse import bacc
    import concourse.mybir as mybir
    from concourse.tile import TileContext

    assert h % block == 0 and block % 32 == 0
    nblk = h // block
    ngrp = block // 32  # 32-row groups per block
    sch = block // 2 + 2  # chunk columns per x tile
    f32 = mybir.dt.float32
    bf16 = mybir.dt.bfloat16

    nc = bacc.Bacc("TRN2", target_bir_lowering=False, debug=False)
    x = nc.dram_tensor("x", [bpc, IN_F, 2, h // 2, CW], bf16, kind="ExternalInput")
    wm = nc.dram_tensor("wm", [128, 6, 128], bf16, kind="ExternalInput")
    wl = nc.dram_tensor("wl", [128, 3, 64], bf16, kind="ExternalInput")
    # [b, group, half(2 uu each), partition, 2048 bf16] — each half-group DMA
    # writes one fully contiguous 512 KB span
    out = nc.dram_tensor(
        "out", [bpc, h // 32, 2, 128, 2048], bf16, kind="ExternalOutput"
    )

    with TileContext(nc) as tc:
        with (
            tc.tile_pool(name="xp", bufs=4) as xpool,
            tc.tile_pool(name="wpool", bufs=1) as wpool,
            tc.tile_pool(name="st", bufs=5) as spool,
            tc.tile_pool(name="ps", bufs=4, space="PSUM") as ppool,
        ):
            wmt = wpool.tile([128, 6, 128], bf16, tag="wm")
            wlt = wpool.tile([128, 3, 64], bf16, tag="wl")
            nc.sync.dma_start(out=wmt[:], in_=wm.ap())
            nc.sync.dma_start(out=wlt[:], in_=wl.ap())
            for b in range(bpc):
                for blk in range(nblk):
                    R = blk * block
                    h0 = R // 2
                    xt = xpool.tile([128, sch, CW], bf16, tag="xt")
                    # host pre-pads rows to 258 with zero borders, so every
                    # transfer is one contiguous span per channel.
                    # half A <- even x rows (chunks 1..sch-1)
                    # half B <- odd x rows (chunks 0..sch-2)
                    if blk == nblk - 1:
                        na = sch - 2  # A chunks 1..sch-2; chunk sch-1 is zero
                        nc.gpsimd.memset(xt[0:64, sch - 1, :], 0.0)
                    else:
                        na = sch - 1
                    if blk == 0:
                        nc.gpsimd.memset(xt[64:128, 0, :], 0.0)
                        b_s, b_lo = 1, 0
                        nb = sch - 2  # B chunks 1..sch-2
                    else:
                        b_s, b_lo = 0, h0 - 1
                        nb = sch - 1  # B chunks 0..sch-2
                    # interleave A/B sub-loads so the first matmuls (which
                    # need both parity halves) unblock as early as possible
                    if b == 0 and blk == 0:
                        cuts = [3, 8, 14, 22]
                    else:
                        cuts = [sch // 2]
                    splits = [0] + cuts + [max(na, nb)]
                    for lo, hi in zip(splits[:-1], splits[1:]):
                        alo, ahi = min(lo, na), min(hi, na)
                        if ahi > alo:
                            nc.sync.dma_start(
                                out=xt[0:64, 1 + alo : 1 + ahi, :],
                                in_=x.ap()[b, :, 0, h0 + alo : h0 + ahi, :],
                            )
                        blo, bhi = min(lo, nb), min(hi, nb)
                        if bhi > blo:
                            nc.sync.dma_start(
                                out=xt[64:128, b_s + blo : b_s + bhi, :],
                                in_=x.ap()[b, :, 1, b_lo + blo : b_lo + bhi, :],
                            )
                    for g in range(ngrp):
                        G = (R + 32 * g) // 32
                        st = spool.tile([128, 4, 2, 2, W], bf16, tag="st")
                        ps = [
                            (
                                ppool.tile([128, 2, W], f32, tag="ps1", name="ps1"),
                                ppool.tile([128, 2, W], f32, tag="ps2", name="ps2"),
                            )
                            for _ in range(4)
                        ]
                        # mains: full-array K=128 M=128 matmuls, both banks
                        for uu in range(4):
                            s0 = 16 * g + 4 * uu + 1
                            ps1, ps2 = ps[uu]
                            for kw in range(3):
                                nc.tensor.matmul(
                                    ps1[:], wmt[:, kw, :],
                                    xt[:, s0 : s0 + 2, kw : kw + W],
                                    start=kw == 0, stop=False,
                                )
                            for kw in range(3):
                                nc.tensor.matmul(
                                    ps2[:], wmt[:, 3 + kw, :],
                                    xt[:, s0 + 2 : s0 + 4, kw : kw + W],
                                    start=kw == 0, stop=False,
                                )
                        # leftovers: 4 distinct PE quadrants per kw-slot
                        for uu in range(4):
                            s0 = 16 * g + 4 * uu + 1
                            ps1, ps2 = ps[uu]
                            for kw in range(3):
                                sp_ = kw == 2
                                nc.tensor.matmul(
                                    ps1[0:64], wlt[64:128, kw, :],
                                    xt[64:128, s0 - 1 : s0 + 1, kw : kw + W],
                                    start=False, stop=sp_,
                                )
                                nc.tensor.matmul(
                                    ps1[64:128], wlt[0:64, kw, :],
                                    xt[0:64, s0 + 1 : s0 + 3, kw : kw + W],
                                    start=False, stop=sp_,
                                )
                                nc.tensor.matmul(
                                    ps2[64:128], wlt[64:128, kw, :],
                                    xt[64:128, s0 + 1 : s0 + 3, kw : kw + W],
                                    start=False, stop=sp_,
                                )
                                nc.tensor.matmul(
                                    ps2[0:64], wlt[0:64, kw, :],
                                    xt[0:64, s0 + 3 : s0 + 5, kw : kw + W],
                                    start=False, stop=sp_,
                                )
                        # full-width PSUM->SBUF copies, split across engines
                        for uu in range(4):
                            ps1, ps2 = ps[uu]
                            nc.scalar.copy(st[:, uu, 0], ps1[:])
                            nc.vector.tensor_copy(out=st[:, uu, 1], in_=ps2[:])
                        nc.gpsimd.dma_start(out=out.ap()[b, G, 0], in_=st[:, 0:2])
                        nc.gpsimd.dma_start(out=out.ap()[b, G, 1], in_=st[:, 2:4])
    nc.compile()
    return nc


def normalize_weight(weight):
    """Host-side equalized-lr + demodulation of the [O,I,3,3] weight."""
    w = np.asarray(weight, dtype=np.float32) * np.float32(C_EQ)
    sigma_inv = 1.0 / np.sqrt(
        np.sum((w * w).astype(np.float32), axis=(1, 2, 3), keepdims=True) + EPS
    )
    return (w * sigma_inv.astype(np.float32)).astype(np.float32)


def pack_weights(w_norm):
    """Pack normalized [O,I,kh,kw] weights into wm [128,6,128] / wl [128,3,64].

    wm column set kw is the merged main tap for bank ps1 (stationary rows
    0:64 act on even x rows / rows 64:128 on odd x rows; output cols 0:64
    are E rows, 64:128 O rows); set 3+kw is the parity-flipped bank ps2.
    wl holds the leftover taps: rows 0:64 = kh=2 (O leftover, moving half
    A), rows 64:128 = kh=0 (E leftover, moving half B).
    """
    wt = np.transpose(w_norm, (2, 3, 1, 0))  # [kh, kw, in, out]
    wm = np.zeros((128, 6, 128), dtype=np.float32)
    wl = np.zeros((128, 3, 64), dtype=np.float32)
    for kw in range(3):
        wm[0:64, kw, 0:64] = wt[1, kw]
        wm[64:128, kw, 0:64] = wt[2, kw]
        wm[0:64, kw, 64:128] = wt[0, kw]
        wm[64:128, kw, 64:128] = wt[1, kw]
        wm[0:64, 3 + kw, 0:64] = wt[0, kw]
        wm[64:128, 3 + kw, 0:64] = wt[1, kw]
        wm[0:64, 3 + kw, 64:128] = wt[1, kw]
        wm[64:128, 3 + kw, 64:128] = wt[2, kw]
        wl[0:64, kw] = wt[2, kw]
        wl[64:128, kw] = wt[0, kw]
    return wm, wl


_NC_CACHE = {}


def _get_nc(bpc, h, block=64):
    key = (bpc, h, block)
    if key not in _NC_CACHE:
        _NC_CACHE[key] = build_nc(bpc, h, block)
    return _NC_CACHE[key]


def split_parity(x_f32):
    """[b, c, h, w] f32 -> bf16 [b, c, 2, h//2, w+2]: row parity split plus
    zero border columns (p=0 even rows, p=1 odd rows)."""
    import ml_dtypes

    b, c, h, w = x_f32.shape
    xb = x_f32.astype(ml_dtypes.bfloat16)
    xP = np.zeros((b, c, 2, h // 2, w + 2), dtype=ml_dtypes.bfloat16)
    xP[:, :, 0, :, 1:-1] = xb[:, :, 0::2]
    xP[:, :, 1, :, 1:-1] = xb[:, :, 1::2]
    return xP


def decode_out(o, h):
    """Device out [bpc, h//32, 2, 128, 2048] bf16 -> fp32 [bpc, 64, h, 256].

    Element (b, G, half, ptop*64+ch, ((uuh*2+bank)*2+j)*256+w) is output
    row 32G + 16half + 8uuh + off, where off = 2j+ptop for bank 0
    (ps1: E rows on partitions 0:64, O on 64:128) and 2j+5-ptop for
    bank 1 (ps2 parity-flipped).
    """
    bpc = o.shape[0]
    nG = h // 32
    ov = np.asarray(o).reshape(bpc, nG, 2, 2, 64, 2, 2, 2, W)
    # dims: (b, G, half, ptop, ch, uuh, bank, j, w)
    outf = np.empty((bpc, OUT_F, h, W), dtype=np.float32)
    outv = outf.reshape(bpc, OUT_F, nG, 2, 2, 8, W)  # (b,ch,G,half,uuh,off,w)
    for ptop in range(2):
        for bank in range(2):
            for j in range(2):
                off = 2 * j + (ptop if bank == 0 else 5 - ptop)
                outv[:, :, :, :, :, off, :] = ov[
                    :, :, :, ptop, :, :, bank, j, :
                ].transpose(0, 3, 1, 2, 4, 5)
    return outf


def kernel(x, weight):
    import ml_dtypes
    from concourse import bass_utils

    x = np.asarray(x, dtype=np.float32)
    weight = np.asarray(weight, dtype=np.float32)
    assert x.shape == (B_FULL, IN_F, H_FULL, W), x.shape

    xP = split_parity(x)
    wm, wl = pack_weights(normalize_weight(weight))
    wm = wm.astype(ml_dtypes.bfloat16)
    wl = wl.astype(ml_dtypes.bfloat16)
    bpc = B_FULL // N_CORES
    nc = _get_nc(bpc, H_FULL)
    in_maps = [
        {"x": xP[i * bpc : (i + 1) * bpc], "wm": wm, "wl": wl}
        for i in range(N_CORES)
    ]
    res = bass_utils.run_bass_kernel_spmd(nc, in_maps, core_ids=list(range(N_CORES)))
    return np.concatenate(
        [decode_out(r["out"], H_FULL) for r in res.results], axis=0
    )
